# revision 1
# baseline (speedup 1.0000x reference)
"""Trainium2 Bass kernel for nn_Encoder (2-layer, B=4 M=16 T=256 D=128 H=8).

Sharding: 64 (b,m) slots -> 8 per core (slot f = b*16+m, core c owns
[8c, 8c+8)). Launch A does LN1/QKV/context-attn/time-attn per slot on
device; relational attention over m (cheap, needs all slots) runs on host
between launches; launch B does Wo/residual/LN2/FFN on device.
"""

import math
import numpy as np

import concourse.bacc as bacc
import concourse.bass as bass
import concourse.tile as tile
from concourse import mybir
from concourse.bass2jax import (
    install_neuronx_cc_hook,
    _bass_exec_p,
    partition_id_tensor,
)

N_CORES = 8
P = 128
T = 256
D = 128
H = 8
DK = 16
L = 16  # context window
SLOTS = 8  # per core
F32 = mybir.dt.float32
F32R = mybir.dt.float32r


def _new_nc():
    return bacc.Bacc(
        "TRN2",
        target_bir_lowering=False,
        debug=False,
        enable_asserts=False,
        num_devices=N_CORES,
    )


def _ap(t, offset, pattern):
    return bass.AP(tensor=t.tensor, offset=t.offset + offset, ap=pattern)


def _bc(row_ap, p):
    # broadcast a [1, N] AP across p partitions
    a = row_ap.copy()
    return bass.AP(tensor=a.tensor, offset=a.offset, ap=[[0, p]] + list(a.ap[1:]))


def build_A():
    nc = _new_nc()
    inp = {}
    for name, shape in [
        ("y", [SLOTS * T, D]),
        ("WqT", [D, D]), ("WkT", [D, D]), ("WvT", [D, D]),
        ("bq", [D, 1]), ("bk", [D, 1]),
        ("bqbc", [P, D]), ("bkbc", [P, D]), ("bvbc", [P, D]),
        ("g1bc", [P, D]), ("b1bc", [P, D]),
        ("mask", [P, 144]),
        ("padcnt", [1, T]),
        ("ident", [P, P]),
        ("pmask0", [P, 1]), ("pmask1", [P, 1]),
        ("ones1", [P, 1]), ("ones8", [P, 8]),
    ]:
        inp[name] = nc.dram_tensor(name, shape, F32, kind="ExternalInput")
    out_xa = nc.dram_tensor("xa", [SLOTS * T, D], F32, kind="ExternalOutput")

    with tile.TileContext(nc) as tc:
        with (
            tc.tile_pool(name="const", bufs=1) as const,
            tc.tile_pool(name="work", bufs=3) as work,
            tc.tile_pool(name="big", bufs=2) as bigp,
            tc.tile_pool(name="ps_small", bufs=1, space="PSUM") as pss,
            tc.tile_pool(name="ps_big", bufs=1, space="PSUM") as psb,
        ):
            C = {}
            for name in ["WqT", "WkT", "WvT"]:
                t = const.tile([D, D], F32, name=f"sb_{name}", tag=f"sb_{name}")
                nc.sync.dma_start(out=t[:], in_=inp[name][:])
                tr = const.tile([D, D], F32R, name=f"sbr_{name}", tag=f"sbr_{name}")
                nc.vector.tensor_copy(out=tr[:], in_=t[:])
                C[name] = tr
            for name, shape in [
                ("bq", [D, 1]), ("bk", [D, 1]),
                ("bqbc", [P, D]), ("bkbc", [P, D]), ("bvbc", [P, D]),
                ("g1bc", [P, D]), ("b1bc", [P, D]),
                ("mask", [P, 144]), ("padcnt", [1, T]), ("ident", [P, P]),
                ("pmask0", [P, 1]), ("pmask1", [P, 1]), ("ones8", [P, 8]),
            ]:
                t = const.tile(shape, F32, name=f"sb_{name}", tag=f"sb_{name}")
                nc.sync.dma_start(out=t[:], in_=inp[name][:])
                C[name] = t
            ones1 = const.tile([P, 1], F32, name="sb_ones1")
            nc.sync.dma_start(out=ones1[:], in_=inp["ones1"][:])
            ones1r = const.tile([P, 1], F32R, name="sbr_ones1")
            nc.vector.tensor_copy(out=ones1r[:], in_=ones1[:])
            eps = const.tile([P, 1], F32, name="eps")
            nc.vector.memset(eps[:], 1e-6)

            for s in range(SLOTS):
                # ---- load + LN1 (token-partition tiles) ----
                hn = []
                for q in range(2):
                    yt = work.tile([P, D], F32, name=f"y_{s}_{q}", tag="y")
                    nc.sync.dma_start(
                        out=yt[:], in_=inp["y"][s * T + q * P : s * T + (q + 1) * P, :]
                    )
                    st = work.tile([P, 6], F32, name=f"st_{s}_{q}", tag="st")
                    nc.vector.bn_stats(out=st[:], in_=yt[:])
                    mv = work.tile([P, 2], F32, name=f"mv_{s}_{q}", tag="mv")
                    nc.vector.bn_aggr(out=mv[:], in_=st[:])
                    sd = work.tile([P, 1], F32, name=f"sd_{s}_{q}", tag="sd")
                    nc.scalar.activation(
                        out=sd[:], in_=mv[:, 1:2],
                        func=mybir.ActivationFunctionType.Sqrt,
                        bias=eps[:], scale=1.0,
                    )
                    rs = work.tile([P, 1], F32, name=f"rs_{s}_{q}", tag="rs")
                    nc.vector.reciprocal(out=rs[:], in_=sd[:])
                    hh = work.tile([P, D], F32, name=f"h_{s}_{q}", tag="h")
                    nc.vector.tensor_scalar(
                        out=hh[:], in0=yt[:], scalar1=mv[:, 0:1], scalar2=rs[:],
                        op0=mybir.AluOpType.subtract, op1=mybir.AluOpType.mult,
                    )
                    nc.vector.tensor_mul(hh[:], hh[:], C["g1bc"][:])
                    nc.vector.tensor_add(hh[:], hh[:], C["b1bc"][:])
                    hn.append(hh)

                # ---- transpose h -> hT [d, t] f32r ----
                hT = work.tile([D, T], F32R, name=f"hT_{s}", tag="hT")
                for q in range(2):
                    tp = pss.tile([P, P], F32, name=f"tp_{s}_{q}", tag="tp32")
                    nc.tensor.transpose(tp[:], hn[q][:], C["ident"][:])
                    nc.vector.tensor_copy(out=hT[:, q * P : (q + 1) * P], in_=tp[:])

                # ---- QKV ----
                # transposed layouts qT,kT [d,t] with per-partition bias
                tl = {}
                for nm, w, b in [("q", "WqT", "bq"), ("k", "WkT", "bk")]:
                    ps = pss.tile([D, T], F32, name=f"ps{nm}T_{s}", tag="psT")
                    nc.tensor.matmul(ps[:], C[w][:], hT[:], start=True, stop=True)
                    zt = work.tile([D, T], F32R, name=f"{nm}T_{s}", tag="tlT", bufs=3)
                    nc.vector.tensor_scalar_add(zt[:], ps[:], C[b][:])
                    tl[nm] = zt
                # token layouts q_B,k_B,v_B [t,d] (+bias broadcast)
                tok = {}
                for nm, w, bb in [("q", "WqT", "bqbc"), ("k", "WkT", "bkbc"),
                                  ("v", "WvT", "bvbc")]:
                    halves = []
                    for q in range(2):
                        ps = pss.tile([P, D], F32, name=f"ps{nm}B_{s}_{q}", tag="tp32")
                        nc.tensor.matmul(
                            ps[:], hT[:, q * P : (q + 1) * P],
                            C[w][:], start=True, stop=True,
                        )
                        if nm == "v":
                            zb = work.tile([P, D], F32, name=f"{nm}B_{s}_{q}", tag="vB", bufs=3)
                        else:
                            zb = work.tile([P, D], F32R, name=f"{nm}B_{s}_{q}", tag="qkB", bufs=6)
                        nc.vector.tensor_add(zb[:], ps[:], C[bb][:])
                        halves.append(zb)
                    tok[nm] = halves

                # ---- context attention on q and k -> cqT/ckT [d,t] f32r ----
                ctx = {}
                sc = 1.0 / math.sqrt(D)
                for nm in ["q", "k"]:
                    zT = tl[nm]
                    num = pss.tile([D, T], F32, name=f"num_{s}_{nm}", tag="num")
                    den = pss.tile([1, T], F32, name=f"den_{s}_{nm}", tag="den")
                    for o in range(2):
                        w = 144 if o == 0 else 128
                        sp = pss.tile([P, 144], F32, name=f"ctxS_{s}_{nm}_{o}", tag="ctxS")
                        nc.tensor.matmul(
                            sp[:, :w],
                            zT[:, o * P : (o + 1) * P],
                            zT[:, o * P : o * P + w],
                            start=True, stop=True,
                        )
                        ex = work.tile([P, 144], F32, name=f"ctxE_{s}_{nm}_{o}", tag="ctxE")
                        nc.scalar.activation(
                            out=ex[:, :w], in_=sp[:, :w],
                            func=mybir.ActivationFunctionType.Exp, scale=sc,
                        )
                        em = work.tile([P, 144], F32R, name=f"ctxM_{s}_{nm}_{o}", tag="ctxM")
                        nc.vector.tensor_mul(em[:, :w], ex[:, :w], C["mask"][:, :w])
                        nc.tensor.matmul(
                            num[:, o * P : o * P + w],
                            tok[nm][o][:],
                            em[:, :w],
                            start=(o == 0), stop=(o == 1),
                        )
                        nc.tensor.matmul(
                            den[:, o * P : o * P + w],
                            ones1r[:],
                            em[:, :w],
                            start=(o == 0), stop=(o == 1),
                        )
                    dn = work.tile([1, T], F32, name=f"dn_{s}_{nm}", tag="dn")
                    nc.vector.tensor_add(dn[:], den[:], C["padcnt"][:])
                    nc.vector.reciprocal(out=dn[:], in_=dn[:])
                    dnb = work.tile([P, T], F32, name=f"dnb_{s}_{nm}", tag="dnb", bufs=2)
                    nc.gpsimd.partition_broadcast(dnb[:], dn[:])
                    cT = work.tile([D, T], F32R, name=f"c{nm}T_{s}", tag="cT", bufs=3)
                    nc.vector.tensor_tensor(
                        out=cT[:], in0=num[:], in1=dnb[:],
                        op=mybir.AluOpType.mult,
                    )
                    ctx[nm] = cT

                # parity-zeroed copies of cqT (rhs of score matmuls)
                cqp = []
                for par in range(2):
                    t = work.tile([D, T], F32R, name=f"cqp_{s}_{par}", tag="cqp")
                    nc.vector.tensor_scalar_mul(
                        t[:], ctx["q"][:], C[f"pmask{par}"][:]
                    )
                    cqp.append(t)

                # ---- time attention ----
                e2 = []
                for kh in range(2):
                    ee = bigp.tile([P, 2048], F32, name=f"e2_{s}_{kh}", tag="e2", bufs=2)
                    for hg in range(2):
                        s2 = psb.tile([P, 1024], F32, name=f"s2_{s}_{kh}_{hg}", tag="s2")
                        for hi in range(4):
                            h = hg * 4 + hi
                            st32 = h // 2
                            par = h % 2
                            kw = dict()
                            if st32 == 3:
                                kw["tile_position"] = (96, 0)
                            nc.tensor.matmul(
                                s2[:, hi * T : (hi + 1) * T],
                                ctx["k"][32 * st32 : 32 * st32 + 32,
                                         kh * P : (kh + 1) * P],
                                cqp[par][32 * st32 : 32 * st32 + 32, :],
                                start=True, stop=True, **kw,
                            )
                        nc.scalar.activation(
                            out=ee[:, hg * 1024 : (hg + 1) * 1024], in_=s2[:],
                            func=mybir.ActivationFunctionType.Exp, scale=0.25,
                        )
                    e2.append(ee)

                # VX: v columns interleaved with ones (denominator trick)
                vx = []
                for kh in range(2):
                    t = work.tile([P, 136], F32, name=f"vx_{s}_{kh}", tag="vx")
                    t3 = t[:].rearrange("p (h c) -> p h c", c=17)
                    nc.vector.tensor_copy(
                        out=t3[:, :, 0:16],
                        in_=tok["v"][kh][:].rearrange("p (h c) -> p h c", c=16),
                    )
                    nc.vector.tensor_copy(
                        out=t3[:, :, 16:17],
                        in_=C["ones8"][:].rearrange("p (h o) -> p h o", o=1),
                    )
                    vx.append(t)

                for qh in range(2):
                    xap = pss.tile([P, 136], F32, name=f"xap_{s}_{qh}", tag="xap")
                    for h in range(H):
                        for kh in range(2):
                            nc.tensor.matmul(
                                xap[:, 17 * h : 17 * h + 17],
                                e2[kh][:, h * T + qh * P : h * T + (qh + 1) * P],
                                vx[kh][:, 17 * h : 17 * h + 17],
                                start=(kh == 0), stop=(kh == 1),
                            )
                    xap3 = xap[:].rearrange("p (h c) -> p h c", c=17)
                    dd = work.tile([P, 8], F32, name=f"dd_{s}_{qh}", tag="dd")
                    nc.vector.tensor_copy(
                        out=dd[:].rearrange("p (h o) -> p h o", o=1),
                        in_=xap3[:, :, 16:17],
                    )
                    nc.vector.reciprocal(out=dd[:], in_=dd[:])
                    xo = work.tile([P, D], F32, name=f"xo_{s}_{qh}", tag="xo")
                    ddb = dd[:].rearrange("p (h o) -> p h o", o=1).broadcast_to((P, 8, 16))
                    nc.vector.tensor_tensor(
                        out=xo[:].rearrange("p (h c) -> p h c", c=16),
                        in0=xap3[:, :, 0:16],
                        in1=ddb,
                        op=mybir.AluOpType.mult,
                    )
                    nc.sync.dma_start(
                        out=out_xa[s * T + qh * P : s * T + (qh + 1) * P, :],
                        in_=xo[:],
                    )
    nc.compile()
    return nc


def build_B():
    nc = _new_nc()
    inp = {}
    for name, shape in [
        ("xrT", [SLOTS * D, T]),
        ("y", [SLOTS * T, D]),
        ("WoT", [D, D]), ("bo", [D, 1]),
        ("W1T", [D, 4 * D]), ("b1", [4 * D, 1]),
        ("W2T", [4 * D, D]), ("b2", [D, 1]),
        ("g2bc", [P, D]), ("b2bc", [P, D]),
        ("ident", [P, P]),
    ]:
        inp[name] = nc.dram_tensor(name, shape, F32, kind="ExternalInput")
    out_y = nc.dram_tensor("yo", [SLOTS * T, D], F32, kind="ExternalOutput")

    with tile.TileContext(nc) as tc:
        with (
            tc.tile_pool(name="const", bufs=1) as const,
            tc.tile_pool(name="work", bufs=3) as work,
            tc.tile_pool(name="ps", bufs=1, space="PSUM") as pss,
        ):
            C = {}
            WoT = const.tile([D, D], F32, name="sb_WoT")
            nc.sync.dma_start(out=WoT[:], in_=inp["WoT"][:])
            WoTr = const.tile([D, D], F32R, name="sbr_WoT")
            nc.vector.tensor_copy(out=WoTr[:], in_=WoT[:])
            W1T = const.tile([D, 4 * D], F32, name="sb_W1T")
            nc.sync.dma_start(out=W1T[:], in_=inp["W1T"][:])
            W1Tr = const.tile([D, 4 * D], F32R, name="sbr_W1T")
            nc.vector.tensor_copy(out=W1Tr[:], in_=W1T[:])
            W2T = []
            for j in range(4):
                t = const.tile([P, D], F32, name=f"sb_W2T_{j}")
                nc.sync.dma_start(out=t[:], in_=inp["W2T"][j * P : (j + 1) * P, :])
                W2T.append(t)
            b1s = []
            for j in range(4):
                t = const.tile([P, 1], F32, name=f"sb_b1_{j}")
                nc.sync.dma_start(out=t[:], in_=inp["b1"][j * P : (j + 1) * P, :])
                b1s.append(t)
            for name, shape in [
                ("bo", [D, 1]), ("b2", [D, 1]),
                ("g2bc", [P, D]), ("b2bc", [P, D]), ("ident", [P, P]),
            ]:
                t = const.tile(shape, F32, name=f"sb_{name}", tag=f"sb_{name}")
                nc.sync.dma_start(out=t[:], in_=inp[name][:])
                C[name] = t
            eps = const.tile([P, 1], F32, name="eps")
            nc.vector.memset(eps[:], 1e-6)

            for s in range(SLOTS):
                xrT = work.tile([D, T], F32, name=f"xrT_{s}", tag="xrT")
                nc.sync.dma_start(
                    out=xrT[:], in_=inp["xrT"][s * D : (s + 1) * D, :]
                )
                xrTr = work.tile([D, T], F32R, name=f"xrTr_{s}", tag="xrTr")
                nc.vector.tensor_copy(out=xrTr[:], in_=xrT[:])
                aps = pss.tile([D, T], F32, name=f"aps_{s}", tag="aps")
                nc.tensor.matmul(aps[:], WoTr[:], xrTr[:], start=True, stop=True)
                zT = work.tile([D, T], F32, name=f"zT_{s}", tag="zT")
                nc.vector.tensor_scalar_add(zT[:], aps[:], C["bo"][:])

                y2h = []
                for q in range(2):
                    tp = pss.tile([P, P], F32, name=f"tpz_{s}_{q}", tag="tpz")
                    nc.tensor.transpose(
                        tp[:], zT[:, q * P : (q + 1) * P], C["ident"][:]
                    )
                    yt = work.tile([P, D], F32, name=f"yin_{s}_{q}", tag="yin")
                    nc.sync.dma_start(
                        out=yt[:],
                        in_=inp["y"][s * T + q * P : s * T + (q + 1) * P, :],
                    )
                    y1 = work.tile([P, D], F32, name=f"y1_{s}_{q}", tag="y1")
                    nc.vector.tensor_add(y1[:], yt[:], tp[:])
                    y2h.append(y1)

                # LN2 + transpose
                h2T = work.tile([D, T], F32R, name=f"h2T_{s}", tag="h2T")
                for q in range(2):
                    y1 = y2h[q]
                    st = work.tile([P, 6], F32, name=f"st2_{s}_{q}", tag="st2")
                    nc.vector.bn_stats(out=st[:], in_=y1[:])
                    mv = work.tile([P, 2], F32, name=f"mv2_{s}_{q}", tag="mv2")
                    nc.vector.bn_aggr(out=mv[:], in_=st[:])
                    sd = work.tile([P, 1], F32, name=f"sd2_{s}_{q}", tag="sd2")
                    nc.scalar.activation(
                        out=sd[:], in_=mv[:, 1:2],
                        func=mybir.ActivationFunctionType.Sqrt,
                        bias=eps[:], scale=1.0,
                    )
                    rs = work.tile([P, 1], F32, name=f"rs2_{s}_{q}", tag="rs2")
                    nc.vector.reciprocal(out=rs[:], in_=sd[:])
                    hh = work.tile([P, D], F32, name=f"h2_{s}_{q}", tag="h2")
                    nc.vector.tensor_scalar(
                        out=hh[:], in0=y1[:], scalar1=mv[:, 0:1], scalar2=rs[:],
                        op0=mybir.AluOpType.subtract, op1=mybir.AluOpType.mult,
                    )
                    nc.vector.tensor_mul(hh[:], hh[:], C["g2bc"][:])
                    nc.vector.tensor_add(hh[:], hh[:], C["b2bc"][:])
                    tp = pss.tile([P, P], F32, name=f"tph2_{s}_{q}", tag="tph2")
                    nc.tensor.transpose(tp[:], hh[:], C["ident"][:])
                    nc.vector.tensor_copy(out=h2T[:, q * P : (q + 1) * P], in_=tp[:])

                # FFN
                gs = []
                for j in range(4):
                    f1 = pss.tile([P, T], F32, name=f"f1_{s}_{j}", tag="f1", bufs=2)
                    nc.tensor.matmul(
                        f1[:], W1Tr[:, j * P : (j + 1) * P], h2T[:],
                        start=True, stop=True,
                    )
                    g = work.tile([P, T], F32, name=f"g_{s}_{j}", tag="g")
                    nc.scalar.activation(
                        out=g[:], in_=f1[:],
                        func=mybir.ActivationFunctionType.Relu,
                        bias=b1s[j][:], scale=1.0,
                    )
                    gs.append(g)
                f2 = pss.tile([D, T], F32, name=f"f2_{s}", tag="f2")
                for j in range(4):
                    nc.tensor.matmul(
                        f2[:], W2T[j][:], gs[j][:],
                        start=(j == 0), stop=(j == 3),
                    )
                f2b = work.tile([D, T], F32, name=f"f2b_{s}", tag="f2b")
                nc.vector.tensor_scalar_add(f2b[:], f2[:], C["b2"][:])
                for q in range(2):
                    tp = pss.tile([P, P], F32, name=f"tpf_{s}_{q}", tag="tpf")
                    nc.tensor.transpose(
                        tp[:], f2b[:, q * P : (q + 1) * P], C["ident"][:]
                    )
                    yo = work.tile([P, D], F32, name=f"yo_{s}_{q}", tag="yo")
                    nc.vector.tensor_add(yo[:], y2h[q][:], tp[:])
                    nc.sync.dma_start(
                        out=out_y[s * T + q * P : s * T + (q + 1) * P, :],
                        in_=yo[:],
                    )
    nc.compile()
    return nc


def _make_runner(nc, n_cores):
    import jax
    from jax.sharding import Mesh, PartitionSpec
    from jax.experimental.shard_map import shard_map

    install_neuronx_cc_hook()
    partition_name = nc.partition_id_tensor.name if nc.partition_id_tensor else None
    in_names, out_names, out_avals, zero_outs = [], [], [], []
    for alloc in nc.m.functions[0].allocations:
        if not isinstance(alloc, mybir.MemoryLocationSet):
            continue
        name = alloc.memorylocations[0].name
        if alloc.kind == "ExternalInput":
            if name != partition_name:
                in_names.append(name)
        elif alloc.kind == "ExternalOutput":
            shape = tuple(alloc.tensor_shape)
            dtype = mybir.dt.np(alloc.dtype)
            out_names.append(name)
            out_avals.append(jax.core.ShapedArray(shape, dtype))
            zero_outs.append(np.zeros(shape, dtype))
    n_params = len(in_names)
    all_in = list(in_names) + list(out_names)
    if partition_name is not None:
        all_in.append(partition_name)
    donate = tuple(range(n_params, n_params + len(out_names)))

    def _body(*args):
        operands = list(args)
        if partition_name is not None:
            operands.append(partition_id_tensor())
        return tuple(
            _bass_exec_p.bind(
                *operands,
                out_avals=tuple(out_avals),
                in_names=tuple(all_in),
                out_names=tuple(out_names),
                lowering_input_output_aliases=(),
                sim_require_finite=False,
                sim_require_nnan=False,
                nc=nc,
            )
        )

    devices = jax.devices()[:n_cores]
    mesh = Mesh(np.asarray(devices), ("core",))
    sharded = jax.jit(
        shard_map(
            _body,
            mesh=mesh,
            in_specs=(PartitionSpec("core"),) * (n_params + len(out_names)),
            out_specs=(PartitionSpec("core"),) * len(out_names),
            check_rep=False,
        ),
        donate_argnums=donate,
        keep_unused=True,
    )

    def run(in_maps):
        concat_in = [
            np.concatenate([np.asarray(m[nm]) for m in in_maps], axis=0)
            for nm in in_names
        ]
        concat_zeros = [
            np.zeros((n_cores * z.shape[0], *z.shape[1:]), z.dtype)
            for z in zero_outs
        ]
        outs = sharded(*concat_in, *concat_zeros)
        outs = [np.asarray(a) for a in outs]
        return [
            {
                nm: outs[i].reshape(n_cores, *out_avals[i].shape)[c]
                for i, nm in enumerate(out_names)
            }
            for c in range(n_cores)
        ]

    return run


_CACHE = {}


def _runners():
    if "A" not in _CACHE:
        _CACHE["A"] = _make_runner(build_A(), N_CORES)
        _CACHE["B"] = _make_runner(build_B(), N_CORES)
    return _CACHE["A"], _CACHE["B"]


def kernel(x, Wq, bq, Wk, bk, Wv, bv, Wo, bo, W1, b1, W2, b2,
           ln1_g, ln1_b, ln2_g, ln2_b, lnf_g, lnf_b, context_len):
    x = np.asarray(x, np.float32)
    B, M, Tt, Dd = x.shape
    assert (B, M, Tt, Dd) == (4, 16, 256, 128) and int(context_len) == 16
    runA, runB = _runners()

    to32 = lambda a: np.asarray(a, np.float32)
    ident = np.eye(P, dtype=np.float32)
    mask = np.zeros((P, 144), np.float32)
    for p in range(P):
        mask[p, p : p + 16] = 1.0
    padcnt = np.maximum(0, 15 - np.arange(T)).astype(np.float32)[None, :]
    pm0 = np.zeros((P, 1), np.float32)
    pm1 = np.zeros((P, 1), np.float32)
    for h in range(H):
        (pm0 if h % 2 == 0 else pm1)[h * DK : (h + 1) * DK] = 1.0
    ones1 = np.ones((P, 1), np.float32)
    ones8 = np.ones((P, 8), np.float32)

    y = x.reshape(64, T, D).copy()  # slot f = b*16+m
    for li in range(Wq.shape[0]):
        WqT = to32(Wq[li]).T.copy()
        WkT = to32(Wk[li]).T.copy()
        WvT = to32(Wv[li]).T.copy()
        common = dict(
            WqT=WqT, WkT=WkT, WvT=WvT,
            bq=to32(bq[li])[:, None], bk=to32(bk[li])[:, None],
            bqbc=np.broadcast_to(to32(bq[li]), (P, D)).copy(),
            bkbc=np.broadcast_to(to32(bk[li]), (P, D)).copy(),
            bvbc=np.broadcast_to(to32(bv[li]), (P, D)).copy(),
            g1bc=np.broadcast_to(to32(ln1_g[li]), (P, D)).copy(),
            b1bc=np.broadcast_to(to32(ln1_b[li]), (P, D)).copy(),
            mask=mask, padcnt=padcnt, ident=ident,
            pmask0=pm0, pmask1=pm1, ones1=ones1, ones8=ones8,
        )
        in_maps = [
            dict(common, y=y[8 * c : 8 * c + 8].reshape(SLOTS * T, D))
            for c in range(N_CORES)
        ]
        res = runA(in_maps)
        xa = np.concatenate(
            [r["xa"].reshape(SLOTS, T, D) for r in res], axis=0
        )  # [64, t, d]

        # host relational attention over m + torch permute/view scramble
        xa5 = xa.reshape(B, M, T, H, DK)
        s2 = np.einsum("bqthd,bkthd->bthqk", xa5, xa5) / math.sqrt(DK)
        s2 = s2 - s2.max(axis=-1, keepdims=True)
        w = np.exp(s2)
        w /= w.sum(axis=-1, keepdims=True)
        xr = np.einsum("bthqk,bkthd->bthqd", w, xa5)
        xr = np.transpose(xr, (3, 0, 1, 2, 4)).reshape(B, M, T, D)
        xrT = np.transpose(xr.reshape(64, T, D), (0, 2, 1)).copy()  # [64, d, t]

        commonB = dict(
            WoT=to32(Wo[li]).T.copy(), bo=to32(bo[li])[:, None],
            W1T=to32(W1[li]).T.copy(), b1=to32(b1[li])[:, None],
            W2T=to32(W2[li]).T.copy(), b2=to32(b2[li])[:, None],
            g2bc=np.broadcast_to(to32(ln2_g[li]), (P, D)).copy(),
            b2bc=np.broadcast_to(to32(ln2_b[li]), (P, D)).copy(),
            ident=ident,
        )
        in_mapsB = [
            dict(
                commonB,
                xrT=xrT[8 * c : 8 * c + 8].reshape(SLOTS * D, T),
                y=y[8 * c : 8 * c + 8].reshape(SLOTS * T, D),
            )
            for c in range(N_CORES)
        ]
        resB = runB(in_mapsB)
        y = np.concatenate(
            [r["yo"].reshape(SLOTS, T, D) for r in resB], axis=0
        )

    mu = y.mean(-1, keepdims=True)
    v = ((y - mu) ** 2).mean(-1, keepdims=True)
    yf = (y - mu) / np.sqrt(v + 1e-6) * to32(lnf_g) + to32(lnf_b)
    return yf.reshape(B, M, T, D).astype(np.float32)

